# revision 1
# baseline (speedup 1.0000x reference)
"""Trainium2 Bass kernel for nn_CubicModelLarge (3-layer cubic-feature MLP).

Strategy: tensor-parallel over the cubic multiplier index i (64 values, 8 per
core).  The cubic expansion is never materialized.  Per layer:

  y[b,o] = W_lin@x + b + sum_t W_sq[o,t] xsq[b,t] + sum_i x[b,i] sum_t W_cu[o,i,t] xsq[b,t]

Rewritten per core c (i in I_c = [8c, 8c+8)):

  H[b,(il,o)] = sum_J F[J,b] * Wcub[J,(il,o)]     (one f32r GEMM, J = 2176 rows)
  y_c[b,o]    = lin[b,o] + sum_il xmac[b,il] * H[b,(il,o)]
  y = AllReduce_c(y_c)

F rows: 2048 rotation products x_a*x_{(a+d)%64} (d=0..31), 64 x rows (carries
the symmetrized W_sq fold, sharded over i via the x_i scaling), 64 gap-32
products (halved).  Rotated copies of xT are built with PE selection matmuls;
products on DVE; the i-contraction is fused scalar_tensor_tensor MACs with
per-partition scalars.  Final layer partials are summed on the host.
"""

import numpy as np

D = 64
B = 1024
NCORES = 8
I_PER = D // NCORES          # 8
OUTS = (64, 64, 10)
NKCHUNK = 16                 # rotation chunks (d pairs)
NB = B // 128                # 8 batch chunks

_CACHE = {}


# ---------------------------------------------------------------- host prep --

def _maps():
    iu, ju = np.triu_indices(D)
    tmap = np.zeros((D, D), np.int64)
    tmap[iu, ju] = np.arange(len(iu))
    tmap[ju, iu] = tmap[iu, ju]
    p = np.arange(128)
    rows_t = np.zeros((NKCHUNK, 128), np.int64)
    for k in range(NKCHUNK):
        d = 2 * k + p // 64
        a = p % 64
        rows_t[k] = tmap[a, (a + d) % D]
    d32_t = tmap[np.arange(D), (np.arange(D) + 32) % D]
    return tmap, rows_t, d32_t


def _prep_layer(W, b, out):
    """-> (wcub [NCORES](2176, I_PER*out), wlin [NCORES](65, out))"""
    _, rows_t, d32_t = _maps()
    W_lin = W[:, :D]
    W_sq = W[:, D:D + 2080]
    W_cu = W[:, D + 2080:].reshape(out, D, 2080)

    iu, ju = np.triu_indices(D)
    w2 = np.zeros((out, D, D), np.float32)
    half = np.where(iu == ju, 1.0, 0.5).astype(np.float32)
    w2[:, iu, ju] = W_sq * half
    w2[:, ju, iu] = W_sq * half

    rt = rows_t.reshape(-1)
    wcubs, wlins = [], []
    for core in range(NCORES):
        I = np.arange(core * I_PER, (core + 1) * I_PER)
        M = I_PER * out
        wcub = np.zeros((17 * 128, M), np.float32)
        blk = W_cu[:, I, :][:, :, rt]                       # (out, I_PER, 2048)
        wcub[:2048] = blk.transpose(2, 1, 0).reshape(2048, M)
        w2blk = w2[:, I, :]                                 # (out, I_PER, 64)
        wcub[2048:2048 + D] = w2blk.transpose(2, 1, 0).reshape(D, M)
        d32blk = W_cu[:, I, :][:, :, d32_t] / 2
        wcub[2048 + D:] = d32blk.transpose(2, 1, 0).reshape(D, M)
        wcubs.append(np.ascontiguousarray(wcub))

        wl = np.zeros((65, out), np.float32)
        if core == 0:
            wl[:D] = W_lin.T
            wl[D] = b
        wlins.append(wl)
    return wcubs, wlins


def _sel_consts():
    """Selection matrices, concatenated (64, (NKCHUNK+2)*128).

    slot k in 0..15: [rot_{2k}; rot_{2k+1}]   sel[c, k*128 + h*64 + a] = (c == (a + 2k + h) % 64)
    slot 16: [rot0; rot0]  (builds xT2)
    slot 17: [rot32; rot32] (first 64 cols used, builds xd32)
    """
    sel = np.zeros((D, (NKCHUNK + 2) * 128), np.float32)
    for k in range(NKCHUNK):
        for p in range(128):
            d = 2 * k + p // 64
            a = p % 64
            sel[(a + d) % D, k * 128 + p] = 1.0
    for p in range(128):
        sel[p % 64, NKCHUNK * 128 + p] = 1.0
        sel[(p % 64 + 32) % D, (NKCHUNK + 1) * 128 + p] = 1.0
    return sel


# ------------------------------------------------------------------ builder --

def _build_module():
    import concourse.bacc as bacc
    import concourse.mybir as mybir
    import concourse.tile as tile

    F32 = mybir.dt.float32
    F32R = mybir.dt.float32r
    MULT = mybir.AluOpType.mult
    ADD = mybir.AluOpType.add

    nc = bacc.Bacc("TRN2", target_bir_lowering=False, num_devices=NCORES, debug=False)

    x_in = nc.dram_tensor("x", [B, D], F32, kind="ExternalInput")
    wcub_in = [
        nc.dram_tensor(f"wcub{li}", [17 * 128, I_PER * OUTS[li]], F32, kind="ExternalInput")
        for li in range(3)
    ]
    wlin_in = [
        nc.dram_tensor(f"wlin{li}", [65, OUTS[li]], F32, kind="ExternalInput")
        for li in range(3)
    ]
    colsel_in = nc.dram_tensor("colsel", [D, I_PER], F32, kind="ExternalInput")
    out_ext = nc.dram_tensor("out", [B, OUTS[2]], F32, kind="ExternalOutput")

    sel_c = nc.inline_tensor(_sel_consts(), name="selc")
    ident_c = nc.inline_tensor(np.eye(128, dtype=np.float32), name="identc")

    with tile.TileContext(nc) as tc:
        with (
            tc.tile_pool(name="wpool", bufs=2) as wpool,
            tc.tile_pool(name="spool", bufs=1) as spool,
            tc.tile_pool(name="xpool", bufs=2) as xpool,
            tc.tile_pool(name="qpool", bufs=1) as qpool,
            tc.tile_pool(name="ypool", bufs=2) as ypool,
            tc.tile_pool(name="ps_rep", bufs=2, space="PSUM") as ps_rep,
            tc.tile_pool(name="ps_h", bufs=3, space="PSUM") as ps_h,
            tc.tile_pool(name="ps_small", bufs=3, space="PSUM") as ps_small,
            tc.tile_pool(name="dpool", bufs=2, space="DRAM") as dpool,
        ):
            sel_sb = spool.tile([D, (NKCHUNK + 2) * 128], F32R, tag="sel")
            nc.sync.dma_start(sel_sb[:], sel_c.ap().bitcast(F32R))
            ident_sb = spool.tile([128, 128], F32, tag="ident")
            nc.sync.dma_start(ident_sb[:], ident_c.ap())
            colsel_sb = spool.tile([D, I_PER], F32R, tag="colsel")
            nc.sync.dma_start(colsel_sb[:], colsel_in.ap().bitcast(F32R))

            HB = 512            # half-batch
            NBH = HB // 128     # 4 chunks per half

            # per-layer weight tiles (DMA'd up front; wpool bufs=2 double-buffers)
            weights = []
            for li in range(3):
                M = I_PER * OUTS[li]
                wcub_sb = wpool.tile([128, NKCHUNK, M], F32R, tag="wcub")
                nc.sync.dma_start(
                    wcub_sb[:],
                    wcub_in[li].ap().bitcast(F32R)[: 16 * 128, :]
                    .rearrange("(k p) m -> p k m", p=128),
                )
                wx_sb = wpool.tile([D, M], F32R, tag="wx")
                nc.sync.dma_start(wx_sb[:], wcub_in[li].ap().bitcast(F32R)[2048:2048 + D, :])
                wd32_sb = wpool.tile([D, M], F32R, tag="wd32")
                nc.sync.dma_start(wd32_sb[:], wcub_in[li].ap().bitcast(F32R)[2048 + D:, :])
                wlin_sb = wpool.tile([65, OUTS[li]], F32R, tag="wlin")
                nc.sync.dma_start(wlin_sb[:], wlin_in[li].ap().bitcast(F32R))
                weights.append((wcub_sb, wx_sb, wd32_sb, wlin_sb))

            # x tiles for layer 0, both halves, straight from the input
            x_half = []
            for h in range(2):
                xs = xpool.tile([128, NBH, D], F32, tag=f"x{h}")
                nc.sync.dma_start(
                    xs[:],
                    x_in.ap()[h * HB:(h + 1) * HB, :]
                    .rearrange("(bc p) f -> p bc f", p=128),
                )
                x_half.append(xs)

            for li in range(3):
                out_l = OUTS[li]
                M = I_PER * out_l
                last = li == 2
                wcub_sb, wx_sb, wd32_sb, wlin_sb = weights[li]
                next_x = [None, None]

                for h in range(2):
                    x_sb = x_half[h]

                    # -- phase A
                    xT_sb = xpool.tile([65, HB], F32R, tag=f"xT{h}")
                    for bc in range(NBH):
                        xTp = ps_small.tile([D, 128], F32, tag="small")
                        nc.tensor.transpose(xTp[:], x_sb[:, bc, :], ident_sb[:])
                        nc.scalar.copy(xT_sb[0:D, bc * 128:(bc + 1) * 128], xTp[:])
                    nc.vector.memset(xT_sb[D:65, :].bitcast(F32), 1.0)

                    xT2_sb = xpool.tile([128, HB], F32, tag=f"xT2{h}")
                    rep00 = ps_rep.tile([128, HB], F32, tag="rep")
                    nc.tensor.matmul(
                        rep00[:], sel_sb[:, NKCHUNK * 128:(NKCHUNK + 1) * 128],
                        xT_sb[0:D, :], start=True, stop=True,
                    )
                    nc.scalar.copy(xT2_sb[:], rep00[:])

                    xd32_sb = xpool.tile([D, HB], F32R, tag=f"xd32{h}")
                    rep32 = ps_rep.tile([128, HB], F32, tag="rep")
                    nc.tensor.matmul(
                        rep32[:], sel_sb[:, (NKCHUNK + 1) * 128:(NKCHUNK + 2) * 128],
                        xT_sb[0:D, :], start=True, stop=True,
                    )
                    nc.vector.tensor_mul(xd32_sb[:], xT2_sb[0:D, :], rep32[0:D, :])

                    # -- phase B
                    xsq = []
                    for k in range(NKCHUNK):
                        rep = ps_rep.tile([128, HB], F32, tag="rep")
                        nc.tensor.matmul(
                            rep[:], sel_sb[:, k * 128:(k + 1) * 128],
                            xT_sb[0:D, :], start=True, stop=True,
                        )
                        xq = qpool.tile([128, HB], F32R, tag=f"xsq{k}h{h}")
                        nc.vector.tensor_mul(xq[:], xT2_sb[:], rep[:])
                        xsq.append(xq)

                    # -- phase C
                    y_sb = ypool.tile([128, NBH, out_l], F32, tag=f"y{h}")
                    if not last:
                        for bc in range(NBH):
                            bs = slice(bc * 128, (bc + 1) * 128)
                            h_ps = ps_h.tile([128, M], F32, tag="h")
                            for k in range(NKCHUNK):
                                nc.tensor.matmul(
                                    h_ps[:], xsq[k][:, bs], wcub_sb[:, k, :],
                                    start=(k == 0), stop=False,
                                )
                            nc.tensor.matmul(h_ps[:], xT_sb[0:D, bs], wx_sb[:], start=False, stop=False)
                            nc.tensor.matmul(h_ps[:], xd32_sb[:, bs], wd32_sb[:], start=False, stop=True)

                            lin_ps = ps_small.tile([128, out_l], F32, tag="small")
                            nc.tensor.matmul(lin_ps[:], xT_sb[0:65, bs], wlin_sb[:], start=True, stop=True)
                            xmac_ps = ps_small.tile([128, I_PER], F32, tag="small")
                            nc.tensor.matmul(xmac_ps[:], xT_sb[0:D, bs], colsel_sb[:], start=True, stop=True)
                            xmac_sb = ypool.tile([128, I_PER], F32, tag="xmac")
                            nc.scalar.copy(xmac_sb[:], xmac_ps[:])

                            nc.scalar.copy(y_sb[:, bc, :], lin_ps[:])
                            for il in range(I_PER):
                                nc.vector.scalar_tensor_tensor(
                                    y_sb[:, bc, :],
                                    h_ps[:, il * out_l:(il + 1) * out_l],
                                    xmac_sb[:, il:il + 1],
                                    y_sb[:, bc, :],
                                    op0=MULT, op1=ADD,
                                )

                        # -- phase D: AllReduce this half
                        y_bounce = dpool.tile([HB, out_l], F32, tag=f"ybounce{h}")
                        y_red = dpool.tile([HB, out_l], F32, tag=f"yred{h}")
                        nc.sync.dma_start(
                            y_bounce[:].rearrange("(bc p) o -> p bc o", p=128), y_sb[:]
                        )
                        nc.gpsimd.collective_compute(
                            "AllReduce",
                            ADD,
                            replica_groups=[list(range(NCORES))],
                            ins=[y_bounce.opt()],
                            outs=[y_red.opt()],
                        )
                        xs = xpool.tile([128, NBH, D], F32, tag=f"x{h}")
                        nc.sync.dma_start(
                            xs[:], y_red[:].rearrange("(bc p) f -> p bc f", p=128)
                        )
                        next_x[h] = xs
                    else:
                        # layer 2: stationary-W GEMM, transpose, MAC
                        h_ps = ps_h.tile([M, HB], F32, tag="h")
                        for k in range(NKCHUNK):
                            nc.tensor.matmul(
                                h_ps[:], wcub_sb[:, k, :], xsq[k][:],
                                start=(k == 0), stop=False,
                            )
                        nc.tensor.matmul(h_ps[:], wx_sb[:], xT_sb[0:D, :], start=False, stop=False)
                        nc.tensor.matmul(h_ps[:], wd32_sb[:], xd32_sb[:], start=False, stop=True)
                        h2_sb = ypool.tile([M, HB], F32, tag=f"h2{h}")
                        nc.scalar.copy(h2_sb[:], h_ps[:])

                        for bc in range(NBH):
                            bs = slice(bc * 128, (bc + 1) * 128)
                            h2t_ps = ps_small.tile([128, M], F32, tag="small")
                            nc.tensor.transpose(h2t_ps[:], h2_sb[:, bs], ident_sb[0:M, 0:M])

                            lin_ps = ps_small.tile([128, out_l], F32, tag="small")
                            nc.tensor.matmul(lin_ps[:], xT_sb[0:65, bs], wlin_sb[:], start=True, stop=True)
                            xmac_ps = ps_small.tile([128, I_PER], F32, tag="small")
                            nc.tensor.matmul(xmac_ps[:], xT_sb[0:D, bs], colsel_sb[:], start=True, stop=True)
                            xmac_sb = ypool.tile([128, I_PER], F32, tag="xmac")
                            nc.scalar.copy(xmac_sb[:], xmac_ps[:])

                            nc.scalar.copy(y_sb[:, bc, :], lin_ps[:])
                            for il in range(I_PER):
                                nc.vector.scalar_tensor_tensor(
                                    y_sb[:, bc, :],
                                    h2t_ps[:, il * out_l:(il + 1) * out_l],
                                    xmac_sb[:, il:il + 1],
                                    y_sb[:, bc, :],
                                    op0=MULT, op1=ADD,
                                )

                        nc.sync.dma_start(
                            out_ext.ap()[h * HB:(h + 1) * HB, :]
                            .rearrange("(bc p) o -> p bc o", p=128),
                            y_sb[:],
                        )

                if not last:
                    x_half = next_x

    nc.compile()
    return nc


# ------------------------------------------------------------------- runner --

def kernel(x, W0, b0, W1, b1, W2, b2):
    from concourse.bass_utils import run_bass_kernel_spmd

    if "nc" not in _CACHE:
        _CACHE["nc"] = _build_module()
    nc = _CACHE["nc"]

    x = np.ascontiguousarray(np.asarray(x, np.float32))
    Ws = [np.asarray(W, np.float32) for W in (W0, W1, W2)]
    bs = [np.asarray(b_, np.float32) for b_ in (b0, b1, b2)]

    wcubs, wlins = {}, {}
    for li in range(3):
        wcubs[li], wlins[li] = _prep_layer(Ws[li], bs[li], OUTS[li])

    in_maps = []
    for core in range(NCORES):
        I = np.arange(core * I_PER, (core + 1) * I_PER)
        colsel = np.zeros((D, I_PER), np.float32)
        colsel[I, np.arange(I_PER)] = 1.0
        m = {"x": x, "colsel": colsel}
        for li in range(3):
            m[f"wcub{li}"] = wcubs[li][core]
            m[f"wlin{li}"] = wlins[li][core]
        in_maps.append(m)

    res = run_bass_kernel_spmd(nc, in_maps, core_ids=list(range(NCORES)))
    out = np.zeros((B, OUTS[2]), np.float32)
    for core in range(NCORES):
        out += res.results[core]["out"]
    return out



# revision 4
# speedup vs baseline: 1.2222x; 1.2222x over previous
"""Trainium2 Bass kernel for nn_CubicModelLarge (3-layer cubic-feature MLP).

Tensor-parallel over the cubic multiplier index i (64 values, 8 per core).
The cubic expansion is never materialized.  Per layer:

  y[b,o] = lin[b,o] + sum_il xmac[b,il] * H[b,(il,o)]
  H[b,(il,o)] = sum_J F[J,b] * wcub[J,(il,o)]     (J = 17*128 rows)

F rows: 2048 rotation products x_a*x_{(a+d)%64} (d=0..31), 64 gap-32
products (weights halved), 64 x rows (carries the symmetrized W_sq fold).

fp16 datapath: weights and F in fp16 (PE 1 cyc/row, DVE 2x mode), PSUM/y
accumulation in fp32.  xT built with XBAR transpose DMAs; rotated copies
of xT built with partition-offset SBUF->SBUF DMAs (no PE rotations, no
PSUM round trip).  lin + per-core column-select (xmac) fused in one small
GEMM.  Epilogue MACs read fp16 H from SBUF.  AllReduce per half-batch
(f32); final layer partials summed on the host.
"""

import numpy as np

D = 64
B = 1024
NCORES = 8
I_PER = D // NCORES          # 8
OUTS = (64, 64, 10)
NK = 16                      # rotation chunks (d pairs)
HB = 512                     # half-batch
NBH = HB // 128              # 4 batch chunks per half

_CACHE = {}


# ---------------------------------------------------------------- host prep --

def _maps():
    iu, ju = np.triu_indices(D)
    tmap = np.zeros((D, D), np.int64)
    tmap[iu, ju] = np.arange(len(iu))
    tmap[ju, iu] = tmap[iu, ju]
    p = np.arange(128)
    rows_t = np.zeros((NK, 128), np.int64)
    for k in range(NK):
        d = 2 * k + p // 64
        a = p % 64
        rows_t[k] = tmap[a, (a + d) % D]
    d32_t = tmap[np.arange(D), (np.arange(D) + 32) % D]
    return tmap, rows_t, d32_t


def _prep_layer(W, b, out):
    """-> (wcub [NCORES](17*128, I_PER*out) f16,
           wl   [NCORES](64, out+I_PER) f16,
           wlb  [NCORES](1, out+I_PER) f16)"""
    _, rows_t, d32_t = _maps()
    W_lin = W[:, :D]
    W_sq = W[:, D:D + 2080]
    W_cu = W[:, D + 2080:].reshape(out, D, 2080)

    iu, ju = np.triu_indices(D)
    w2 = np.zeros((out, D, D), np.float32)
    half = np.where(iu == ju, 1.0, 0.5).astype(np.float32)
    w2[:, iu, ju] = W_sq * half
    w2[:, ju, iu] = W_sq * half

    rt = rows_t.reshape(-1)
    wcubs, wls, wlbs = [], [], []
    for core in range(NCORES):
        I = np.arange(core * I_PER, (core + 1) * I_PER)
        M = I_PER * out
        wcub = np.zeros((17 * 128, M), np.float32)
        blk = W_cu[:, I, :][:, :, rt]                       # (out, I_PER, 2048)
        wcub[:2048] = blk.transpose(2, 1, 0).reshape(2048, M)
        # slot 16: [d32/2 (64 rows) ; symmetrized-W_sq x rows (64 rows)]
        d32blk = W_cu[:, I, :][:, :, d32_t] / 2
        wcub[2048:2048 + D] = d32blk.transpose(2, 1, 0).reshape(D, M)
        w2blk = w2[:, I, :]                                 # (out, I_PER, 64)
        wcub[2048 + D:] = w2blk.transpose(2, 1, 0).reshape(D, M)
        wcubs.append(np.ascontiguousarray(wcub.astype(np.float16)))

        wl = np.zeros((D, out + I_PER), np.float32)
        wlb = np.zeros((1, out + I_PER), np.float32)
        if core == 0:
            wl[:, :out] = W_lin.T
            wlb[0, :out] = b
        wl[I, out + np.arange(I_PER)] = 1.0                 # column select
        wls.append(wl.astype(np.float16))
        wlbs.append(wlb.astype(np.float16))
    return wcubs, wls, wlbs


def make_in_maps(x, Ws, bs):
    wcubs, wls, wlbs = {}, {}, {}
    for li in range(3):
        wcubs[li], wls[li], wlbs[li] = _prep_layer(Ws[li], bs[li], OUTS[li])
    in_maps = []
    for core in range(NCORES):
        m = {"x": np.ascontiguousarray(x, np.float32)}
        for li in range(3):
            m[f"wcub{li}"] = wcubs[li][core]
            m[f"wl{li}"] = wls[li][core]
            m[f"wlb{li}"] = wlbs[li][core]
        in_maps.append(m)
    return in_maps


# ------------------------------------------------------------------ builder --

def _build_module():
    import concourse.bacc as bacc
    import concourse.mybir as mybir
    import concourse.tile as tile

    F32 = mybir.dt.float32
    F16 = mybir.dt.float16
    MULT = mybir.AluOpType.mult
    ADD = mybir.AluOpType.add

    nc = bacc.Bacc("TRN2", target_bir_lowering=False, num_devices=NCORES, debug=False)

    x_in = nc.dram_tensor("x", [B, D], F32, kind="ExternalInput")
    wcub_in = [
        nc.dram_tensor(f"wcub{li}", [17 * 128, I_PER * OUTS[li]], F16, kind="ExternalInput")
        for li in range(3)
    ]
    wl_in = [
        nc.dram_tensor(f"wl{li}", [D, OUTS[li] + I_PER], F16, kind="ExternalInput")
        for li in range(3)
    ]
    wlb_in = [
        nc.dram_tensor(f"wlb{li}", [1, OUTS[li] + I_PER], F16, kind="ExternalInput")
        for li in range(3)
    ]
    out_ext = nc.dram_tensor("out", [B, OUTS[2]], F32, kind="ExternalOutput")

    ident_c = nc.inline_tensor(np.eye(128, dtype=np.float32), name="identc")

    with tile.TileContext(nc) as tc:
        with (
            tc.tile_pool(name="wpool", bufs=1) as wpool,
            tc.tile_pool(name="spool", bufs=1) as spool,
            tc.tile_pool(name="xpool", bufs=2) as xpool,
            tc.tile_pool(name="qpool", bufs=1) as qpool,
            tc.tile_pool(name="ypool", bufs=1) as ypool,
            tc.tile_pool(name="ps_h", bufs=3, space="PSUM") as ps_h,
            tc.tile_pool(name="ps_sm", bufs=2, space="PSUM") as ps_sm,
            tc.tile_pool(name="dpool", bufs=2, space="DRAM") as dpool,
        ):
            ident_sb = spool.tile([128, 128], F32, tag="ident")
            nc.sync.dma_start(ident_sb[:], ident_c.ap())
            ones_sb = spool.tile([1, HB], F16, tag="ones")
            nc.vector.memset(ones_sb[:], 1.0)

            # ---- weights: chunked streams, bulk on sync ring -------------
            weights = []
            for li in range(3):
                M = I_PER * OUTS[li]
                wcub_sb = wpool.tile([128, 17, M], F16, tag=f"wcub{li}")
                view = wcub_in[li].ap().rearrange("(k p) m -> p k m", p=128)
                if li < 2:
                    bounds = (0, 5, 9, 13, 17)
                    for j in range(4):
                        a, bnd = bounds[j], bounds[j + 1]
                        nc.sync.dma_start(wcub_sb[:, a:bnd, :], view[:, a:bnd, :])
                else:
                    nc.sync.dma_start(wcub_sb[:], view)
                wl_sb = wpool.tile([D, OUTS[li] + I_PER], F16, tag=f"wl{li}")
                nc.sync.dma_start(wl_sb[:], wl_in[li].ap())
                wlb_sb = wpool.tile([1, OUTS[li] + I_PER], F16, tag=f"wlb{li}")
                nc.sync.dma_start(wlb_sb[:], wlb_in[li].ap())
                weights.append((wcub_sb, wl_sb, wlb_sb))

            # x for layer 0, both halves
            x_half = []
            for h in range(2):
                xs = xpool.tile([128, NBH, D], F32, tag=f"x{h}")
                nc.scalar.dma_start(
                    xs[:],
                    x_in.ap()[h * HB:(h + 1) * HB, :]
                    .rearrange("(bc p) f -> p bc f", p=128),
                )
                x_half.append(xs)

            for li in range(3):
                out_l = OUTS[li]
                M = I_PER * out_l
                last = li == 2
                wcub_sb, wl_sb, wlb_sb = weights[li]
                next_x = [None, None]

                for h in range(2):
                    x_sb = x_half[h]

                    # -- phase A: fp16 x (duplicated cols) + XBAR transpose
                    x16d = xpool.tile([128, NBH, 128], F16, tag=f"x16d{h}")
                    nc.scalar.copy(x16d[:, :, 0:D], x_sb[:])
                    nc.scalar.copy(x16d[:, :, D:128], x_sb[:])
                    xT2 = xpool.tile([128, HB], F16, tag=f"xT2{h}")
                    for bc in range(NBH):
                        eng = nc.scalar if bc % 2 == 0 else nc.sync
                        eng.dma_start_transpose(
                            xT2[:, bc * 128:(bc + 1) * 128], x16d[:, bc, :]
                        )
                    # gap-32 partner rows + [xd32 ; x] stack
                    x32s = xpool.tile([D, HB], F16, tag=f"x32s{h}")
                    nc.scalar.dma_start(x32s[:], xT2[32:96, :])
                    xw = xpool.tile([128, HB], F16, tag=f"xw{h}")
                    nc.vector.tensor_mul(xw[0:D, :], xT2[0:D, :], x32s[:])
                    nc.sync.dma_start(xw[D:128, :], xT2[0:D, :])

                    # -- phase B: rotated copies via DMA, products on DVE
                    rep = qpool.tile([128, NK, HB], F16, tag=f"rep{h}")
                    for k in range(NK):
                        eng = nc.scalar if k % 2 == 0 else nc.sync
                        eng.dma_start(rep[0:D, k, :], xT2[2 * k:2 * k + D, :])
                        eng.dma_start(rep[D:128, k, :], xT2[2 * k + 1:2 * k + 1 + D, :])
                    xsq = qpool.tile([128, NK, HB], F16, tag=f"xsq{h}")
                    for k in range(NK):
                        nc.vector.tensor_mul(xsq[:, k, :], xT2[:], rep[:, k, :])

                    y_sb = ypool.tile([128, NBH, out_l], F32, tag=f"y{h}")
                    if not last:
                        h_sb = ypool.tile([128, NBH, M], F16, tag=f"hsb{h}")
                        lx_sb = ypool.tile([128, NBH, out_l + I_PER], F32, tag=f"lx{h}")
                        for bc in range(NBH):
                            bs = slice(bc * 128, (bc + 1) * 128)
                            h_ps = ps_h.tile([128, M], F32, tag="h")
                            for k in range(NK):
                                nc.tensor.matmul(
                                    h_ps[:], xsq[:, k, bs], wcub_sb[:, k, :],
                                    start=(k == 0), stop=False,
                                )
                            nc.tensor.matmul(h_ps[:], xw[:, bs], wcub_sb[:, 16, :],
                                             start=False, stop=True)
                            lx_ps = ps_sm.tile([128, out_l + I_PER], F32, tag="lx")
                            nc.tensor.matmul(lx_ps[:], xT2[0:D, bs], wl_sb[:],
                                             start=True, stop=False)
                            nc.tensor.matmul(lx_ps[:], ones_sb[:, bs], wlb_sb[:],
                                             start=False, stop=True)
                            nc.scalar.copy(h_sb[:, bc, :], h_ps[:])
                            nc.scalar.copy(lx_sb[:, bc, :], lx_ps[:])
                            for il in range(I_PER):
                                nc.vector.scalar_tensor_tensor(
                                    y_sb[:, bc, :],
                                    h_sb[:, bc, il * out_l:(il + 1) * out_l],
                                    lx_sb[:, bc, out_l + il:out_l + il + 1],
                                    lx_sb[:, bc, 0:out_l] if il == 0 else y_sb[:, bc, :],
                                    op0=MULT, op1=ADD,
                                )

                        # -- AllReduce this half
                        y_bounce = dpool.tile([HB, out_l], F32, tag=f"yb{h}")
                        y_red = dpool.tile([HB, out_l], F32, tag=f"yr{h}")
                        nc.sync.dma_start(
                            y_bounce[:].rearrange("(bc p) o -> p bc o", p=128), y_sb[:]
                        )
                        nc.gpsimd.collective_compute(
                            "AllReduce",
                            ADD,
                            replica_groups=[list(range(NCORES))],
                            ins=[y_bounce.opt()],
                            outs=[y_red.opt()],
                        )
                        xs = xpool.tile([128, NBH, D], F32, tag=f"x{h}")
                        nc.scalar.dma_start(
                            xs[:], y_red[:].rearrange("(bc p) f -> p bc f", p=128)
                        )
                        next_x[h] = xs
                    else:
                        # layer 2: stationary-W GEMM, transpose, MAC
                        hT_ps = ps_h.tile([M, HB], F32, tag="h")
                        for k in range(NK):
                            nc.tensor.matmul(hT_ps[:], wcub_sb[:, k, :], xsq[:, k, :],
                                             start=(k == 0), stop=False)
                        nc.tensor.matmul(hT_ps[:], wcub_sb[:, 16, :], xw[:],
                                         start=False, stop=True)
                        hT_sb = ypool.tile([M, HB], F32, tag=f"hT{h}")
                        nc.scalar.copy(hT_sb[:], hT_ps[:])

                        h2t_sb = ypool.tile([128, NBH, M], F16, tag=f"h2t{h}")
                        lx_sb = ypool.tile([128, NBH, out_l + I_PER], F32, tag=f"lx2{h}")
                        for bc in range(NBH):
                            bs = slice(bc * 128, (bc + 1) * 128)
                            tr_ps = ps_sm.tile([128, M], F32, tag="tr")
                            nc.tensor.transpose(tr_ps[:], hT_sb[:, bs], ident_sb[0:M, 0:M])
                            lx_ps = ps_sm.tile([128, out_l + I_PER], F32, tag="lx")
                            nc.tensor.matmul(lx_ps[:], xT2[0:D, bs], wl_sb[:],
                                             start=True, stop=False)
                            nc.tensor.matmul(lx_ps[:], ones_sb[:, bs], wlb_sb[:],
                                             start=False, stop=True)
                            nc.scalar.copy(h2t_sb[:, bc, :], tr_ps[:])
                            nc.scalar.copy(lx_sb[:, bc, :], lx_ps[:])
                            for il in range(I_PER):
                                nc.vector.scalar_tensor_tensor(
                                    y_sb[:, bc, :],
                                    h2t_sb[:, bc, il * out_l:(il + 1) * out_l],
                                    lx_sb[:, bc, out_l + il:out_l + il + 1],
                                    lx_sb[:, bc, 0:out_l] if il == 0 else y_sb[:, bc, :],
                                    op0=MULT, op1=ADD,
                                )
                        nc.sync.dma_start(
                            out_ext.ap()[h * HB:(h + 1) * HB, :]
                            .rearrange("(bc p) o -> p bc o", p=128),
                            y_sb[:],
                        )

                if not last:
                    x_half = next_x

    nc.compile()
    return nc


# ------------------------------------------------------------------- runner --

def kernel(x, W0, b0, W1, b1, W2, b2):
    from concourse.bass_utils import run_bass_kernel_spmd

    if "nc" not in _CACHE:
        _CACHE["nc"] = _build_module()
    nc = _CACHE["nc"]

    x = np.ascontiguousarray(np.asarray(x, np.float32))
    Ws = [np.asarray(W, np.float32) for W in (W0, W1, W2)]
    bs = [np.asarray(b_, np.float32) for b_ in (b0, b1, b2)]
    in_maps = make_in_maps(x, Ws, bs)

    res = run_bass_kernel_spmd(nc, in_maps, core_ids=list(range(NCORES)))
    out = np.zeros((B, OUTS[2]), np.float32)
    for core in range(NCORES):
        out += res.results[core]["out"]
    return out


# revision 8
# speedup vs baseline: 1.4159x; 1.1585x over previous
"""Trainium2 Bass kernel for nn_CubicModelLarge (3-layer cubic-feature MLP).

Tensor-parallel over the cubic multiplier index i (64 values, 8 per core).
The cubic expansion is never materialized.  Per layer:

  y[b,o] = lin[b,o] + sum_il xmac[b,il] * H[b,(il,o)]
  H[b,(il,o)] = sum_J F[J,b] * wcub[J,(il,o)]     (J = 17*128 rows)

F rows: 2048 rotation products x_a*x_{(a+d)%64} (d=0..31), 64 gap-32
products (weights halved), 64 x rows (carries the symmetrized W_sq fold).

fp16 datapath: weights and F in fp16 (PE 1 cyc/row, DVE 2x mode), PSUM/y
accumulation in fp32.  xT built with XBAR transpose DMAs; rotated copies
of xT built with partition-offset SBUF->SBUF DMAs (no PE rotations, no
PSUM round trip).  lin + per-core column-select (xmac) fused in one small
GEMM.  Epilogue MACs read fp16 H from SBUF.  AllReduce per half-batch
(f32); final layer partials summed on the host.
"""

import numpy as np

D = 64
B = 1024
NCORES = 8
I_PER = D // NCORES          # 8
OUTS = (64, 64, 10)
NK = 16                      # rotation chunks (d pairs)
HB = 512                     # half-batch
NBH = HB // 128              # 4 batch chunks per half

_CACHE = {}


# ---------------------------------------------------------------- host prep --

def _maps():
    iu, ju = np.triu_indices(D)
    tmap = np.zeros((D, D), np.int64)
    tmap[iu, ju] = np.arange(len(iu))
    tmap[ju, iu] = tmap[iu, ju]
    p = np.arange(128)
    rows_t = np.zeros((NK, 128), np.int64)
    for k in range(NK):
        d = 2 * k + p // 64
        a = p % 64
        rows_t[k] = tmap[a, (a + d) % D]
    d32_t = tmap[np.arange(D), (np.arange(D) + 32) % D]
    return tmap, rows_t, d32_t


def _prep_layer(W, b, out):
    """-> (wcub [NCORES](17*128, I_PER*out) f16,
           wl   [NCORES](64, out+I_PER) f16,
           wlb  [NCORES](1, out+I_PER) f16)"""
    _, rows_t, d32_t = _maps()
    W_lin = W[:, :D]
    W_sq = W[:, D:D + 2080]
    W_cu = W[:, D + 2080:].reshape(out, D, 2080)

    iu, ju = np.triu_indices(D)
    w2 = np.zeros((out, D, D), np.float32)
    half = np.where(iu == ju, 1.0, 0.5).astype(np.float32)
    w2[:, iu, ju] = W_sq * half
    w2[:, ju, iu] = W_sq * half

    rt = rows_t.reshape(-1)
    wcubs, wls, wlbs = [], [], []
    for core in range(NCORES):
        I = np.arange(core * I_PER, (core + 1) * I_PER)
        M = I_PER * out
        wcub = np.zeros((17 * 128, M), np.float32)
        blk = W_cu[:, I, :][:, :, rt]                       # (out, I_PER, 2048)
        wcub[:2048] = blk.transpose(2, 1, 0).reshape(2048, M)
        # slot 16: [d32/2 (64 rows) ; symmetrized-W_sq x rows (64 rows)]
        d32blk = W_cu[:, I, :][:, :, d32_t] / 2
        wcub[2048:2048 + D] = d32blk.transpose(2, 1, 0).reshape(D, M)
        w2blk = w2[:, I, :]                                 # (out, I_PER, 64)
        wcub[2048 + D:] = w2blk.transpose(2, 1, 0).reshape(D, M)
        wflat = wcub.reshape(17, 128, M).transpose(1, 0, 2).reshape(128, 17 * M)
        wcubs.append(np.ascontiguousarray(wflat.astype(np.float16)))

        wl = np.zeros((D, out + I_PER), np.float32)
        wlb = np.zeros((1, out + I_PER), np.float32)
        if core == 0:
            wl[:, :out] = W_lin.T
            wlb[0, :out] = b
        wl[I, out + np.arange(I_PER)] = 1.0                 # column select
        wls.append(wl.astype(np.float16))
        wlbs.append(wlb.astype(np.float16))
    return wcubs, wls, wlbs


def make_in_maps(x, Ws, bs):
    wcubs, wls, wlbs = {}, {}, {}
    for li in range(3):
        wcubs[li], wls[li], wlbs[li] = _prep_layer(Ws[li], bs[li], OUTS[li])
    in_maps = []
    for core in range(NCORES):
        m = {"x": np.ascontiguousarray(x, np.float32)}
        for li in range(3):
            m[f"wcub{li}"] = wcubs[li][core]
            m[f"wl{li}"] = wls[li][core]
            m[f"wlb{li}"] = wlbs[li][core]
        in_maps.append(m)
    return in_maps


# ------------------------------------------------------------------ builder --

def _build_module():
    import concourse.bacc as bacc
    import concourse.mybir as mybir
    import concourse.tile as tile

    F32 = mybir.dt.float32
    F16 = mybir.dt.float16
    MULT = mybir.AluOpType.mult
    ADD = mybir.AluOpType.add

    nc = bacc.Bacc("TRN2", target_bir_lowering=False, num_devices=NCORES, debug=False)

    x_in = nc.dram_tensor("x", [B, D], F32, kind="ExternalInput")
    wcub_in = [
        nc.dram_tensor(f"wcub{li}", [128, 17 * I_PER * OUTS[li]], F16, kind="ExternalInput")
        for li in range(3)
    ]
    wl_in = [
        nc.dram_tensor(f"wl{li}", [D, OUTS[li] + I_PER], F16, kind="ExternalInput")
        for li in range(3)
    ]
    wlb_in = [
        nc.dram_tensor(f"wlb{li}", [1, OUTS[li] + I_PER], F16, kind="ExternalInput")
        for li in range(3)
    ]
    out_ext = nc.dram_tensor("out", [B, OUTS[2]], F32, kind="ExternalOutput")

    ident_c = nc.inline_tensor(np.eye(128, dtype=np.float32), name="identc")
    sel = np.zeros((D, 17 * 128), np.float16)
    for k in range(NK):
        for p in range(128):
            sel[(p % 64 + 2 * k + p // 64) % D, k * 128 + p] = 1.0
    for p in range(128):
        sel[(p % 64 + 32) % D, NK * 128 + p] = 1.0
    sel_c = nc.inline_tensor(sel, name="selc")

    with tile.TileContext(nc) as tc:
        with (
            tc.tile_pool(name="wpool", bufs=1) as wpool,
            tc.tile_pool(name="spool", bufs=1) as spool,
            tc.tile_pool(name="xpool", bufs=2) as xpool,
            tc.tile_pool(name="qpool", bufs=1) as qpool,
            tc.tile_pool(name="ypool", bufs=1) as ypool,
            tc.tile_pool(name="ps_h", bufs=2, space="PSUM") as ps_h,
            tc.tile_pool(name="ps_rp", bufs=2, space="PSUM") as ps_rp,
            tc.tile_pool(name="ps_sm", bufs=2, space="PSUM") as ps_sm,
            tc.tile_pool(name="dpool", bufs=2, space="DRAM") as dpool,
        ):
            ident_sb = spool.tile([128, 128], F32, tag="ident")
            nc.sync.dma_start(ident_sb[:], ident_c.ap())
            ones_sb = spool.tile([1, HB], F16, tag="ones")
            nc.vector.memset(ones_sb[:], 1.0)
            sel_sb = spool.tile([D, 17 * 128], F16, tag="sel")
            nc.sync.dma_start(sel_sb[:], sel_c.ap())
            wu_sb = spool.tile([1, 1], F32, tag="wu")
            nc.vector.memset(wu_sb[:], 0.0)
            wu_a = dpool.tile([1, 1], F32, tag="wua")
            wu_b = dpool.tile([1, 1], F32, tag="wub")
            nc.sync.dma_start(wu_a[:], wu_sb[:])
            nc.gpsimd.collective_compute(
                "AllReduce", ADD,
                replica_groups=[list(range(NCORES))],
                ins=[wu_a.opt()], outs=[wu_b.opt()],
            )

            # ---- weights: chunked streams, bulk on sync ring -------------
            weights = []
            for li in range(3):
                M = I_PER * OUTS[li]
                wcub_sb = wpool.tile([128, 17, M], F16, tag=f"wcub{li}")
                view = wcub_in[li].ap()
                if li < 2:
                    bounds = (0, 5, 9, 13, 17)
                    for j in range(4):
                        a, bnd = bounds[j], bounds[j + 1]
                        nc.sync.dma_start(wcub_sb[:, a:bnd, :], view[:, a * M:bnd * M])
                else:
                    nc.sync.dma_start(wcub_sb[:], view)
                wl_sb = wpool.tile([D, OUTS[li] + I_PER], F16, tag=f"wl{li}")
                nc.sync.dma_start(wl_sb[:], wl_in[li].ap())
                wlb_sb = wpool.tile([1, OUTS[li] + I_PER], F16, tag=f"wlb{li}")
                nc.sync.dma_start(wlb_sb[:], wlb_in[li].ap())
                weights.append((wcub_sb, wl_sb, wlb_sb))

            # x for layer 0, both halves
            x_half = []
            for h in range(2):
                xs = xpool.tile([128, NBH, D], F32, tag=f"x{h}")
                nc.scalar.dma_start(
                    xs[:],
                    x_in.ap()[h * HB:(h + 1) * HB, :]
                    .rearrange("(bc p) f -> p bc f", p=128),
                )
                x_half.append(xs)

            for li in range(3):
                out_l = OUTS[li]
                M = I_PER * out_l
                last = li == 2
                wcub_sb, wl_sb, wlb_sb = weights[li]
                next_x = [None, None]

                for h in range(2):
                    x_sb = x_half[h]

                    # -- phase A: PE transpose -> fp16 xT2 (two periods)
                    xT2 = xpool.tile([128, HB], F16, tag=f"xT2{h}")
                    for bc in range(NBH):
                        xtp = ps_sm.tile([D, 128], F32, tag="xtp")
                        nc.tensor.transpose(xtp[:], x_sb[:, bc, :], ident_sb[:])
                        nc.scalar.copy(xT2[0:D, bc * 128:(bc + 1) * 128], xtp[:])
                    nc.sync.dma_start(xT2[D:128, :], xT2[0:D, :])

                    # -- phase B: rotations on PE straight to PSUM,
                    #    fp16 products on DVE read PSUM directly
                    xw = xpool.tile([128, HB], F16, tag=f"xw{h}")
                    xsq = qpool.tile([128, NK, HB], F16, tag=f"xsq{h}")

                    def rot(k):
                        rp = ps_rp.tile([128, HB], F32, tag="rp")
                        nc.tensor.matmul(
                            rp[:], sel_sb[:, k * 128:(k + 1) * 128], xT2[0:D, :],
                            start=True, stop=True,
                        )
                        if k == NK:
                            nc.vector.tensor_mul(xw[0:D, :], xT2[0:D, :], rp[0:D, :])
                        else:
                            nc.vector.tensor_mul(xsq[:, k, :], xT2[:], rp[:])

                    rot(NK)
                    nc.sync.dma_start(xw[D:128, :], xT2[0:D, :])
                    rot(0)
                    rot(1)

                    y_sb = ypool.tile([128, NBH, out_l], F32, tag=f"y{h}")
                    if not last:
                        h_sb = ypool.tile([128, NBH, M], F16, tag=f"hsb{h}")
                        lx_sb = ypool.tile([128, NBH, out_l + I_PER], F32, tag=f"lx{h}")
                        for bc in range(NBH):
                            bs = slice(bc * 128, (bc + 1) * 128)
                            h_ps = ps_h.tile([128, M], F32, tag="h")
                            for k in range(NK):
                                nc.tensor.matmul(
                                    h_ps[:], xsq[:, k, bs], wcub_sb[:, k, :],
                                    start=(k == 0), stop=False,
                                )
                                if bc == 0 and k + 2 < NK:
                                    rot(k + 2)
                            nc.tensor.matmul(h_ps[:], xw[:, bs], wcub_sb[:, 16, :],
                                             start=False, stop=True)
                            lx_ps = ps_sm.tile([128, out_l + I_PER], F32, tag="lx")
                            nc.tensor.matmul(lx_ps[:], xT2[0:D, bs], wl_sb[:],
                                             start=True, stop=False)
                            nc.tensor.matmul(lx_ps[:], ones_sb[:, bs], wlb_sb[:],
                                             start=False, stop=True)
                            nc.scalar.copy(h_sb[:, bc, :], h_ps[:])
                            nc.scalar.copy(lx_sb[:, bc, :], lx_ps[:])
                            for il in range(I_PER):
                                nc.vector.scalar_tensor_tensor(
                                    y_sb[:, bc, :],
                                    h_sb[:, bc, il * out_l:(il + 1) * out_l],
                                    lx_sb[:, bc, out_l + il:out_l + il + 1],
                                    lx_sb[:, bc, 0:out_l] if il == 0 else y_sb[:, bc, :],
                                    op0=MULT, op1=ADD,
                                )

                        # -- AllReduce this half
                        y_bounce = dpool.tile([HB, out_l], F32, tag=f"yb{h}")
                        y_red = dpool.tile([HB, out_l], F32, tag=f"yr{h}")
                        nc.sync.dma_start(
                            y_bounce[:].rearrange("(bc p) o -> p bc o", p=128), y_sb[:]
                        )
                        nc.gpsimd.collective_compute(
                            "AllReduce",
                            ADD,
                            replica_groups=[list(range(NCORES))],
                            ins=[y_bounce.opt()],
                            outs=[y_red.opt()],
                        )
                        xs = xpool.tile([128, NBH, D], F32, tag=f"x{h}")
                        nc.scalar.dma_start(
                            xs[:], y_red[:].rearrange("(bc p) f -> p bc f", p=128)
                        )
                        next_x[h] = xs
                    else:
                        # layer 2: stationary-W GEMM, transpose, MAC
                        hT_ps = ps_h.tile([M, HB], F32, tag="h")
                        for k in range(NK):
                            nc.tensor.matmul(hT_ps[:], wcub_sb[:, k, :], xsq[:, k, :],
                                             start=(k == 0), stop=False)
                            if k + 2 < NK:
                                rot(k + 2)
                        nc.tensor.matmul(hT_ps[:], wcub_sb[:, 16, :], xw[:],
                                         start=False, stop=True)
                        hT_sb = ypool.tile([M, HB], F32, tag=f"hT{h}")
                        nc.scalar.copy(hT_sb[:], hT_ps[:])

                        h2t_sb = ypool.tile([128, NBH, M], F16, tag=f"h2t{h}")
                        lx_sb = ypool.tile([128, NBH, out_l + I_PER], F32, tag=f"lx2{h}")
                        for bc in range(NBH):
                            bs = slice(bc * 128, (bc + 1) * 128)
                            tr_ps = ps_rp.tile([128, M], F32, tag="rp")
                            nc.tensor.transpose(tr_ps[:], hT_sb[:, bs], ident_sb[0:M, 0:M])
                            lx_ps = ps_sm.tile([128, out_l + I_PER], F32, tag="lx")
                            nc.tensor.matmul(lx_ps[:], xT2[0:D, bs], wl_sb[:],
                                             start=True, stop=False)
                            nc.tensor.matmul(lx_ps[:], ones_sb[:, bs], wlb_sb[:],
                                             start=False, stop=True)
                            nc.scalar.copy(h2t_sb[:, bc, :], tr_ps[:])
                            nc.scalar.copy(lx_sb[:, bc, :], lx_ps[:])
                            for il in range(I_PER):
                                nc.vector.scalar_tensor_tensor(
                                    y_sb[:, bc, :],
                                    h2t_sb[:, bc, il * out_l:(il + 1) * out_l],
                                    lx_sb[:, bc, out_l + il:out_l + il + 1],
                                    lx_sb[:, bc, 0:out_l] if il == 0 else y_sb[:, bc, :],
                                    op0=MULT, op1=ADD,
                                )
                        nc.sync.dma_start(
                            out_ext.ap()[h * HB:(h + 1) * HB, :]
                            .rearrange("(bc p) o -> p bc o", p=128),
                            y_sb[:],
                        )

                if not last:
                    x_half = next_x

    nc.compile()
    return nc


# ------------------------------------------------------------------- runner --

def kernel(x, W0, b0, W1, b1, W2, b2):
    from concourse.bass_utils import run_bass_kernel_spmd

    if "nc" not in _CACHE:
        _CACHE["nc"] = _build_module()
    nc = _CACHE["nc"]

    x = np.ascontiguousarray(np.asarray(x, np.float32))
    Ws = [np.asarray(W, np.float32) for W in (W0, W1, W2)]
    bs = [np.asarray(b_, np.float32) for b_ in (b0, b1, b2)]
    in_maps = make_in_maps(x, Ws, bs)

    res = run_bass_kernel_spmd(nc, in_maps, core_ids=list(range(NCORES)))
    out = np.zeros((B, OUTS[2]), np.float32)
    for core in range(NCORES):
        out += res.results[core]["out"]
    return out
